# revision 34
# baseline (speedup 1.0000x reference)
"""Local (windowed) attention kernel for Trainium2, sequence-parallel over 8 NeuronCores.

Reference computation (fp32):
    qkv = x @ w_qkv ; q,k,v split, reshaped to (head, window, 128, 64)
    k,v get a 1-window zero-padded lookback -> (head, window, 256, 64)
    sim = q @ k.T * d^-0.5, causal-banded mask, softmax, out = attn @ v
    y = out @ w_out + b_out

Sharding: 128 windows of 128 tokens -> 16 windows per core, plus a 128-row
halo of x from the previous core (zeros for core 0, which exactly reproduces
the reference's zero-pad lookback including its effect on the softmax
denominator). No inter-core communication.

Device dataflow (per core, all bf16 matmuls accumulating in fp32):
  Phase B: qkT[m] = w_qk[:,m].T @ xT keeps q/k features on partitions.
    Round 0 accumulates in chunk-major contraction half-waves (SBUF joins,
    psum rotating through 6 banks) so the in-order PE rides the input DMA
    stream without head-of-line blocking; round 1 runs as whole chains
    straight after the waves; rounds 2-4 are drip-fed between attention
    steps as filler chains.
  Phase C: v = xT.T @ w_v keeps tokens on partitions, with a ones-column
    appended per head so attn@v also emits the softmax denominator.
  Phase D: scores are computed transposed (pT[j,i]) per k-TILE: one 256-col
    matmul covers both q-windows that attend to that k tile. exp on the Act
    engine (no max-subtraction: logits ~N(0,0.4)); causal mask by one
    batched DVE multiply; attn@v with pT slices as the *stationary* operand
    so the output lands tokens-on-partitions [i, d | denom] and the whole
    softmax normalization is one GPSIMD normalize_recip per head;
    PE-transpose back to [hd, tokens] for the projection.
    NOTE: matmuls with different PE tile_position row offsets (heads with
    q/k features on partitions 0:64 vs 64:128) must not share a PSUM bank
    — each score bank groups two same-offset heads.
  Phase E: y = attnT.T @ w_out, written out in bf16.
Emission is software-pipelined (S/A/T/E stages over k-tiles/windows) and
interleaved with the projection rounds so PE stays busy and Act/DVE/Pool
run in parallel.
"""

import sys

sys.path.insert(0, "/opt/trn_rl_repo")

import numpy as np
import ml_dtypes

import concourse.bass as bass
import concourse.mybir as mybir
import concourse.tile as tile
from concourse import bacc
from concourse.bass_utils import run_bass_kernel_spmd

BF16 = mybir.dt.bfloat16
F32 = mybir.dt.float32

N = 16384
DIM = 1024
HEADS = 8
DHEAD = 64
WSZ = 128
NCORES = 8
R = N // NCORES            # 2048 own rows per core
T = R + WSZ                # 2176 rows incl. halo
NW = R // WSZ              # 16 own windows
DK = DIM // 128            # 8 contraction chunks
P = 128
SCALE = DHEAD ** -0.5

# score-tile head groups: two heads per PSUM bank, same q/k partition offset
SGROUPS = [(0, 2), (1, 3), (4, 6), (5, 7)]
GRP_OF = {h: g for g, hs in enumerate(SGROUPS) for h in hs}
IDX_OF = {h: e for hs in SGROUPS for e, h in enumerate(hs)}

_CACHE = {}


def _build():
    nc = bacc.Bacc()
    xT_d = nc.declare_dram_parameter("xT", [DIM, T], BF16, isOutput=False)
    wqkv_d = nc.declare_dram_parameter("wqkv", [DIM, 3 * HEADS * DHEAD], BF16, isOutput=False)
    wout_d = nc.declare_dram_parameter("wout", [HEADS * DHEAD, DIM], BF16, isOutput=False)
    mask2_d = nc.declare_dram_parameter("mask2", [P, 2, P], BF16, isOutput=False)
    ident_d = nc.declare_dram_parameter("ident", [P, P], BF16, isOutput=False)
    out_d = nc.declare_dram_parameter("out", [R, DIM], BF16, isOutput=True)

    # projection column rounds: q only needs own tokens, k needs the halo too
    qblocks = [(WSZ + r * 512, 512) for r in range(4)]
    kblocks = [(0, 512), (512, 512), (1024, 512), (1536, 512), (2048, 128)]

    with tile.TileContext(nc) as tc:
        with (
            tc.tile_pool(name="pers", bufs=1) as pers,
            # pt tiles are read by A one and two steps after creation; 16
            # bufs (4 steps x 4 tiles) keeps slot reuse strictly behind all
            # already-emitted readers, so each step's exp never waits on a
            # same-step A consumer (fewer bufs reintroduce that stall, and
            # 8 or fewer is a real WAR race on hardware)
            tc.tile_pool(name="ptp", bufs=16) as ptp,
            tc.tile_pool(name="osbp", bufs=6) as osbp,
            tc.tile_pool(name="atokp", bufs=4) as atokp,
            tc.tile_pool(name="eop", bufs=4) as eop,
            tc.tile_pool(name="ps512", bufs=3, space="PSUM") as ps512,
            tc.tile_pool(name="psS", bufs=2, space="PSUM") as psS,
            tc.tile_pool(name="psP", bufs=2, space="PSUM") as psP,
            tc.tile_pool(name="psT", bufs=1, space="PSUM") as psT,
        ):
            # ---- persistent SBUF tiles ------------------------------------
            xT_all = pers.tile([P, DK, T], BF16, tag="xT")
            w_all = pers.tile([P, DK, 3 * HEADS * DHEAD], BF16, tag="w")
            wo_all = pers.tile([P, 4, DIM], BF16, tag="wo")
            xT_sb = [xT_all[:, k, :] for k in range(DK)]
            w_sb = [w_all[:, k, :] for k in range(DK)]
            wo_sb = [wo_all[:, m, :] for m in range(4)]
            qk_sb = [pers.tile([P, T], BF16, tag=f"qk{m}", name=f"qk{m}") for m in range(8)]
            v_sb = [pers.tile([P, HEADS, DHEAD + 1], BF16, tag=f"v{t}", name=f"v{t}") for t in range(NW + 1)]
            attnT_sb = pers.tile([P, 4, R], BF16, tag="attnT")
            mask2_sb = pers.tile([P, 2, P], BF16, tag="mask2")
            ident_sb = pers.tile([P, P], BF16, tag="ident")

            # ---- input DMAs: k-granular, issued from 3 queues -------------
            xT_dv = xT_d.rearrange("(k p) t -> p k t", p=P)
            w_dv = wqkv_d.rearrange("(k p) c -> p k c", p=P)
            wo_dv = wout_d.rearrange("(m p) c -> p m c", p=P)
            # ONE queue in exact consumption order: concurrent queues split
            # HBM bandwidth evenly so no chunk lands early; serial transfers
            # deliver chunk k at ~k/8 of the load time and the first
            # projection waves start ~3x sooner
            # chunk 0 is sliced so the first wave-0 chains (q m0 needs
            # w[0:128]+xT[128:640]; k m4 needs w[512:640]+xT[0:512]) unblock
            # after ~0.4MB instead of the full chunk
            # all slivers on the sync queue: the Act queue's ring wakes later
            # (behind its activation-table load), measured ~1.5us slower to
            # first matmul when the lead slivers ride it
            nc.sync.dma_start(w_all[:, 0, 0:128], w_dv[:, 0, 0:128])
            nc.sync.dma_start(xT_all[:, 0, 0:640], xT_dv[:, 0, 0:640])
            nc.sync.dma_start(w_all[:, 0, 128:768], w_dv[:, 0, 128:768])
            nc.sync.dma_start(w_all[:, 0, 768:1536], w_dv[:, 0, 768:1536])
            nc.sync.dma_start(xT_all[:, 0, 640:1152], xT_dv[:, 0, 640:1152])
            nc.sync.dma_start(xT_all[:, 0, 1152:T], xT_dv[:, 0, 1152:T])
            for k in range(1, DK):
                nc.sync.dma_start(xT_all[:, k, :], xT_dv[:, k, :])
                nc.sync.dma_start(w_all[:, k, :], w_dv[:, k, :])
            nc.sync.dma_start(wo_all[:], wo_dv[:])
            nc.sync.dma_start(mask2_sb[:], mask2_d[:])
            nc.sync.dma_start(ident_sb[:], ident_d[:])
            for t in range(NW + 1):
                nc.vector.memset(v_sb[t][:, :, DHEAD:DHEAD + 1], 1.0)

            # ---- projection emitters --------------------------------------
            # ks = (k0, k1): contraction chunk range; join='copy' writes the
            # half-sum, join='add' accumulates onto the existing SBUF value.
            def emit_B(m, blk, ks=(0, DK), join="copy", eng="v", pool=None,
                       ptag="mm512"):
                b0, bw = blk
                pl = ps512 if pool is None else pool
                pq = pl.tile([P, 512], F32, tag=ptag, name="pq")
                for k in range(*ks):
                    nc.tensor.matmul(
                        pq[:, :bw],
                        lhsT=w_sb[k][:, m * P:(m + 1) * P],
                        rhs=xT_sb[k][:, b0:b0 + bw],
                        start=(k == ks[0]), stop=(k == ks[1] - 1),
                    )
                dst = qk_sb[m][:, b0:b0 + bw]
                if join == "copy":
                    if eng == "v":
                        nc.vector.tensor_copy(dst, pq[:, :bw])
                    else:
                        nc.scalar.copy(dst, pq[:, :bw])
                else:
                    nc.vector.tensor_add(dst, pq[:, :bw], dst)

            def emit_C(t, ks=(0, DK), join="copy", eng="s", pool=None,
                       ptag="mm512"):
                pl = ps512 if pool is None else pool
                pv = pl.tile([P, 512], F32, tag=ptag, name="pv")
                for k in range(*ks):
                    nc.tensor.matmul(
                        pv[:],
                        lhsT=xT_sb[k][:, t * P:(t + 1) * P],
                        rhs=w_sb[k][:, 1024:1536],
                        start=(k == ks[0]), stop=(k == ks[1] - 1),
                    )
                dst = v_sb[t][:, :, 0:DHEAD]
                src = pv.rearrange("p (h d) -> p h d", h=HEADS)
                if join != "copy":
                    nc.vector.tensor_add(dst, src, dst)
                elif eng == "v":
                    nc.vector.tensor_copy(dst, src)
                else:
                    nc.scalar.copy(dst, src)

            # projection chains of round r as thunks, in a PE-friendly order
            def round_B(r, ks=(0, DK), join="copy", keng="s", qeng="v"):
                ch = []
                for m in range(4):
                    if r < 4:
                        ch.append(lambda m=m: emit_B(m, qblocks[r], ks, join, eng=qeng))
                    ch.append(lambda m=m: emit_B(4 + m, kblocks[r], ks, join, eng=keng))
                return ch

            def round_C(r, ks=(0, DK), join="copy", eng="s"):
                if r < 4:
                    return [(lambda t=t: emit_C(t, ks, join, eng))
                            for t in range(4 * r, 4 * r + 4)]
                return [lambda: emit_C(NW, ks, join, eng)]

            def round_chains(r, ks=(0, DK), join="copy", eng="s"):
                return round_B(r, ks, join, keng=eng) + round_C(r, ks, join, eng)

            # ---- attention emitters ---------------------------------------
            # S: transposed scores for k-tile t vs the (up to) two q-windows
            # that attend to it, for head group g. pt[:, e, 0:128] serves
            # window t-1 (current-window, masked), pt[:, e, 128:256] serves
            # window t (lookback, unmasked).
            def emit_S(t, g):
                off = (g % 2) * DHEAD
                q0 = t * P + (P if t == 0 else 0)
                qw = P if (t == 0 or t == NW) else 2 * P
                u0 = P if t == 0 else 0
                ps = psS.tile([P, 2, 2 * P], F32, tag="sc", name="ps")
                for e, h in enumerate(SGROUPS[g]):
                    a = h // 2
                    nc.tensor.matmul(
                        ps[:, e, u0:u0 + qw],
                        lhsT=qk_sb[4 + a][off:off + DHEAD, t * P:(t + 1) * P],
                        rhs=qk_sb[a][off:off + DHEAD, q0:q0 + qw],
                        start=True, stop=True,
                    )
                pt = ptp.tile([P, 2, 2 * P], BF16, tag="pt", name="pt")
                nc.scalar.activation(pt[:, :, u0:u0 + qw], ps[:, :, u0:u0 + qw],
                                     mybir.ActivationFunctionType.Exp, scale=SCALE)
                # causal mask of the current-window half (q-window t-1)
                if t > 0:
                    nc.vector.tensor_mul(pt[:, :, 0:P], pt[:, :, 0:P], mask2_sb[:])
                return pt

            # A: attn@v for heads 4b..4b+3 of window w, tokens-on-partitions
            # + GPSIMD normalize. pts maps k-tile index -> group tiles.
            def emit_A_mms(w, b, ptsw, ptsw1):
                po = psP.tile([P, 4, P], F32, tag="po", name="po")
                mms = []
                for hh in range(4):
                    h = 4 * b + hh
                    g, e = GRP_OF[h], IDX_OF[h]
                    # jc0: lookback half of tile w (cols 128:256)
                    mms.append(lambda hh=hh, g=g, e=e, h=h: nc.tensor.matmul(
                        po[:, hh, 0:DHEAD + 1],
                        lhsT=ptsw[g][:, e, P:2 * P],
                        rhs=v_sb[w][:, h, :],
                        start=True, stop=False,
                    ))
                    # jc1: current-window half of tile w+1 (cols 0:128)
                    mms.append(lambda hh=hh, g=g, e=e, h=h: nc.tensor.matmul(
                        po[:, hh, 0:DHEAD + 1],
                        lhsT=ptsw1[g][:, e, 0:P],
                        rhs=v_sb[w + 1][:, h, :],
                        start=False, stop=True,
                    ))
                return po, mms

            def emit_A_join(w, b, po, atok):
                osb = osbp.tile([P, 4, DHEAD + 1], F32, tag="osb", name="osb")
                nc.vector.tensor_copy(osb[:], po[:, :, 0:DHEAD + 1])
                for hh in range(4):
                    nc.gpsimd.normalize_recip(
                        atok[:, 4 * b + hh, :],
                        osb[:, hh, 0:DHEAD],
                        osb[:, hh, DHEAD:DHEAD + 1],
                    )

            def emit_A(w, b, ptsw, ptsw1, atok):
                po, mms = emit_A_mms(w, b, ptsw, ptsw1)
                for mm in mms:
                    mm()
                emit_A_join(w, b, po, atok)

            # T: PE-transpose normalized attn back to [hd, tokens].
            def emit_T(w, atok):
                pT = psT.tile([P, 4, P], F32, tag="tr", name="pT")
                for m in range(4):
                    nc.tensor.matmul(
                        pT[:, m, :],
                        lhsT=atok[:, 2 * m:2 * m + 2, :],
                        rhs=ident_sb[:],
                        start=True, stop=True,
                    )
                nc.vector.tensor_copy(attnT_sb[:, :, w * P:(w + 1) * P], pT[:])

            # E: output projection, one 512-wide half at a time. The four
            # matmul thunks are returned so the caller can interleave them
            # with attn@v matmuls (whose 128-row LDWEIGHTS then hides under
            # these 512-col streams instead of stalling the PE).
            def emit_E_mms(w, nf, pool=None, ptag="mm512"):
                pl = ps512 if pool is None else pool
                pf = pl.tile([P, 512], F32, tag=ptag, name="pf")
                mms = [
                    (lambda m=m: nc.tensor.matmul(
                        pf[:],
                        lhsT=attnT_sb[:, m, w * P:(w + 1) * P],
                        rhs=wo_sb[m][:, nf * 512:(nf + 1) * 512],
                        start=(m == 0), stop=(m == 3),
                    )) for m in range(4)
                ]
                return pf, mms

            def emit_E_join(w, nf, eo, pf):
                if nf == 0:
                    nc.scalar.copy(eo[:, 0:512], pf[:])
                else:
                    nc.vector.tensor_copy(eo[:, 512:1024], pf[:])
                nc.sync.dma_start(out_d[w * P:(w + 1) * P, nf * 512:(nf + 1) * 512],
                                  eo[:, nf * 512:(nf + 1) * 512])

            def emit_E(w):
                eo = eop.tile([P, DIM], BF16, tag="eo", name="eo")
                for nf in range(2):
                    pf, mms = emit_E_mms(w, nf)
                    for mm in mms:
                        mm()
                    if w == NW - 1 and nf == 1:
                        # the very last copy+store is the kernel's critical
                        # tail: run it as two parallel halves on Act+DVE with
                        # stores issued from two queues
                        nc.scalar.copy(eo[:, 512:768], pf[:, 0:256])
                        nc.vector.tensor_copy(eo[:, 768:1024], pf[:, 256:512])
                        nc.sync.dma_start(out_d[w * P:(w + 1) * P, 512:768],
                                          eo[:, 512:768])
                        nc.scalar.dma_start(out_d[w * P:(w + 1) * P, 768:1024],
                                            eo[:, 768:1024])
                    else:
                        emit_E_join(w, nf, eo, pf)

            # ---- software-pipelined schedule ------------------------------
            # Round 0 rides the input DMA stream as chunk-major half-waves:
            # 6 chains (matching the 6 rotating PSUM slots below) emit all
            # their chunk-k matmuls before any chunk-k+1 matmul, so the
            # in-order PE never head-of-line blocks ready chunk-k work behind
            # a not-yet-landed chunk. Waves (0,1),(1,2),(2,5),(5,8) keep the
            # SBUF-accumulate joins at 3 adds per chain (DVE adds cost
            # ~600ns; 36 of them just fit inside the stream window). Wave-0
            # copies split DVE/Act. Round 1 runs as whole contraction chains
            # straight after the waves: by then every chunk has landed (or is
            # about to), giving the PE a 20us backlog that covers the join
            # drain and the stream tail; its copies go to Act, which is
            # otherwise idle until the first exp.
            # Wave psum tiles rotate through 6 banks (ps512's 3 plus the
            # attention pools psP/psT, idle until the stream ends).
            slotv = [(ps512, "mm512"), (ps512, "mm512"), (ps512, "mm512"),
                     (psP, "po"), (psP, "po"), (psT, "tr")]
            sc = [0]
            wspec = []
            for m in range(4):
                wspec.append(("B", m))
                wspec.append(("B", 4 + m))
            for t in range(4):
                wspec.append(("C", t))

            def emit_wave(ks, join, specs, eng_of):
                tiles = []
                for _ in specs:
                    pool, ptag = slotv[sc[0] % 6]
                    sc[0] += 1
                    tiles.append(pool.tile([P, 512], F32, tag=ptag, name="pwv"))
                for k in range(*ks):
                    for (kind, a), pq in zip(specs, tiles):
                        if kind == "B":
                            b0, bw = qblocks[0] if a < 4 else kblocks[0]
                            nc.tensor.matmul(
                                pq[:, :bw],
                                lhsT=w_sb[k][:, a * P:(a + 1) * P],
                                rhs=xT_sb[k][:, b0:b0 + bw],
                                start=(k == ks[0]), stop=(k == ks[1] - 1),
                            )
                        else:
                            nc.tensor.matmul(
                                pq[:],
                                lhsT=xT_sb[k][:, a * P:(a + 1) * P],
                                rhs=w_sb[k][:, 1024:1536],
                                start=(k == ks[0]), stop=(k == ks[1] - 1),
                            )
                for ci, ((kind, a), pq) in enumerate(zip(specs, tiles)):
                    eng = eng_of(ci)
                    if kind == "B":
                        b0, bw = qblocks[0] if a < 4 else kblocks[0]
                        dst = qk_sb[a][:, b0:b0 + bw]
                        if join == "copy":
                            if eng == "v":
                                nc.vector.tensor_copy(dst, pq[:, :bw])
                            else:
                                nc.scalar.copy(dst, pq[:, :bw])
                        else:
                            nc.vector.tensor_add(dst, pq[:, :bw], dst)
                    else:
                        dst = v_sb[a][:, :, 0:DHEAD]
                        src = pq.rearrange("p (h d) -> p h d", h=HEADS)
                        if join == "copy":
                            if eng == "v":
                                nc.vector.tensor_copy(dst, src)
                            else:
                                nc.scalar.copy(dst, src)
                        else:
                            nc.vector.tensor_add(dst, src, dst)

            def nslot():
                pool, ptag = slotv[sc[0] % 6]
                sc[0] += 1
                return pool, ptag

            # r1 is split so ready work always sits between chunk-gated wave
            # matmuls (the in-order PE otherwise head-of-line blocks): the
            # first 6 r1 chains run chunks 0:5 (landed) between waves 2 and
            # 3, their 5:8 tails run right after wave 3, and the rest of r1
            # runs as whole chains once every chunk is down.
            def emit_r1(chs, ks, join):
                for kind, a in chs:
                    pool, ptag = nslot()
                    eng = "v" if join == "add" else "s"
                    if kind == "B":
                        blk = qblocks[1] if a < 4 else kblocks[1]
                        emit_B(a, blk, ks, join, eng=eng, pool=pool, ptag=ptag)
                    else:
                        emit_C(a, ks, join, eng=eng, pool=pool, ptag=ptag)

            r1spec = []
            for m in range(4):
                r1spec.append(("B", m))
                r1spec.append(("B", 4 + m))
            for t in range(4, 8):
                r1spec.append(("C", t))

            alt_vs = lambda ci: "v" if ci % 2 == 0 else "s"
            dve = lambda ci: "v"
            for half in (wspec[0:6], wspec[6:12]):
                emit_wave((0, 1), "copy", half, alt_vs)
            for half in (wspec[0:6], wspec[6:12]):
                emit_wave((1, 2), "add", half, dve)
            # six r1 chains over the already-landed chunks plug the PE's
            # wait for chunk 2 at the wave-1 -> wave-2 boundary (sized for
            # the slowest observed DMA ramp, ~1.9us)
            emit_r1(r1spec[0:6], (0, 2), "copy")
            for ks in [(2, 5), (5, 8)]:
                for half in (wspec[0:6], wspec[6:12]):
                    emit_wave(ks, "add", half, dve)
            emit_r1(r1spec[0:6], (2, DK), "add")
            emit_r1(r1spec[6:12], (0, DK), "copy")
            # filler chains per step, respecting readiness deadlines:
            # r2 by step 7, r3-B by step 11, C(t) by step t, r4 k-tail by
            # step 15; late C chains pad the filler-less tail.
            r3C = round_C(3)   # C(12..15)
            r4B = round_B(4)
            r4C = round_C(4)   # C(16)
            step_fill = {i: [] for i in range(NW + 1)}
            r2 = round_chains(2)
            r3B = round_B(3)
            for n, ch in enumerate(r2):             # deadline: step 7
                step_fill[n // 2].append(ch)
            for n, ch in enumerate(r3B):            # deadline: step 11
                step_fill[6 + (n + 1) // 2].append(ch)
            step_fill[11].append(r3C[0])
            step_fill[12].append(r3C[1])
            step_fill[13].extend([r3C[2], r4B[0], r4B[1]])
            step_fill[14].extend([r3C[3], r4B[2], r4B[3]])
            step_fill[15].append(r4C[0])

            pts_of = {}
            atok_of = {}
            for i in range(NW):
                fl = list(step_fill.get(i, []))
                tS, wA = i, i - 2
                if 0 <= wA <= NW - 1:
                    atok_of[wA] = atokp.tile([P, HEADS, DHEAD], BF16, tag="atok", name="atok")
                if fl:
                    fl.pop(0)()
                pts = [emit_S(tS, 0), emit_S(tS, 1)]
                if fl:
                    fl.pop(0)()
                if wA in atok_of:
                    emit_A(wA, 0, pts_of[wA], pts_of[wA + 1], atok_of[wA])
                pts += [emit_S(tS, 2), emit_S(tS, 3)]
                if fl:
                    fl.pop(0)()
                if wA in atok_of:
                    emit_A(wA, 1, pts_of[wA], pts_of[wA + 1], atok_of[wA])
                pts_of[tS] = pts
                if tS == NW - 1:
                    # pull the last score tile into this step: its exp then
                    # completes a step early and the final window's whole
                    # A->normalize->T->E chain starts sooner
                    pts_of[NW] = [emit_S(NW, g) for g in range(4)]
                for ch in fl:
                    ch()
                if wA - 1 in pts_of and wA - 1 >= 0:
                    del pts_of[wA - 1]
                if i - 3 in atok_of:
                    emit_T(i - 3, atok_of[i - 3])
                    del atok_of[i - 3]
                if 0 <= i - 4 <= NW - 1:
                    emit_E(i - 4)

            # compressed drain: the final windows' A bursts (65-col matmuls,
            # LDWEIGHTS-bound) are woven between the remaining E projections'
            # 512-col streams so the PE stays fed, and the last copies and
            # stores are split across engines and DMA queues to shorten the
            # post-compute tail. Weave keeps each A (start,stop) PSUM pair
            # adjacent and slots one E matmul between pairs.
            def weave(amms, emms):
                ia = 0
                for mm in emms:
                    for _ in range(2):
                        if ia < len(amms):
                            amms[ia]()
                            ia += 1
                    mm()
                while ia < len(amms):
                    amms[ia]()
                    ia += 1

            # drain-window E joins skip the per-half sync-queue stores: one
            # full-row store per window (halved descriptor count) issued from
            # the Act/GpSimd queues keeps the sync queue empty for the last
            # window's latency-critical sliver stores.
            # both drain copies on Act: the Vector queue is strict-FIFO and a
            # 600ns CAST there stalls the short transpose copies that gate
            # the next window's E matmuls
            def drain_E_join(w, nf, eo, pf):
                nc.scalar.copy(eo[:, nf * 512:(nf + 1) * 512], pf[:])

            # last window's normalize on DVE (reciprocal + per-partition
            # scalar multiplies): the GpSimd queue's ~0.5us/op dispatch chain
            # would gate the final transpose; DVE is nearly idle here
            def emit_A_join_dve(w, b, po, atok):
                osb = osbp.tile([P, 4, DHEAD + 1], F32, tag="osb", name="osb")
                nc.vector.tensor_copy(osb[:], po[:, :, 0:DHEAD + 1])
                nc.vector.reciprocal(osb[:, :, DHEAD:DHEAD + 1],
                                     osb[:, :, DHEAD:DHEAD + 1])
                for hh in range(4):
                    nc.vector.tensor_scalar_mul(
                        atok[:, 4 * b + hh, :],
                        osb[:, hh, 0:DHEAD],
                        osb[:, hh, DHEAD:DHEAD + 1],
                    )

            atok_of[NW - 2] = atokp.tile([P, HEADS, DHEAD], BF16, tag="atok", name="atok")
            atok_of[NW - 1] = atokp.tile([P, HEADS, DHEAD], BF16, tag="atok", name="atok")
            # d-step 1: A(NW-2) woven with E(NW-4), then T(NW-3)
            eoA = eop.tile([P, DIM], BF16, tag="eo", name="eo")
            for b in range(2):
                po, amms = emit_A_mms(NW - 2, b, pts_of[NW - 2], pts_of[NW - 1])
                pf, emms = emit_E_mms(NW - 4, b)
                weave(amms, emms)
                emit_A_join(NW - 2, b, po, atok_of[NW - 2])
                drain_E_join(NW - 4, b, eoA, pf)
            nc.scalar.dma_start(out_d[(NW - 4) * P:(NW - 3) * P, :], eoA[:, :])
            emit_T(NW - 3, atok_of[NW - 3])
            # d-step 2: A(NW-1) woven with E(NW-3), then T(NW-2)
            eoB = eop.tile([P, DIM], BF16, tag="eo", name="eo")
            for b in range(2):
                po, amms = emit_A_mms(NW - 1, b, pts_of[NW - 1], pts_of[NW])
                pf, emms = emit_E_mms(NW - 3, b)
                weave(amms, emms)
                emit_A_join_dve(NW - 1, b, po, atok_of[NW - 1])
                drain_E_join(NW - 3, b, eoB, pf)
            # sync queue (idle here) carries this store: a gpsimd-queue store
            # would sit in FIFO between the normalizes that gate T(NW-1)
            nc.sync.dma_start(out_d[(NW - 3) * P:(NW - 2) * P, :], eoB[:, :])
            # T(NW-2) with the attnT copy split DVE/Act: E(NW-2)'s first two
            # matmuls unblock after the half-copy instead of the full 690ns
            pT2 = psT.tile([P, 4, P], F32, tag="tr", name="pT2")
            for m in range(4):
                nc.tensor.matmul(
                    pT2[:, m, :],
                    lhsT=atok_of[NW - 2][:, 2 * m:2 * m + 2, :],
                    rhs=ident_sb[:],
                    start=True, stop=True,
                )
            nc.vector.tensor_copy(attnT_sb[:, 0:2, (NW - 2) * P:(NW - 1) * P],
                                  pT2[:, 0:2, :])
            nc.scalar.copy(attnT_sb[:, 2:4, (NW - 2) * P:(NW - 1) * P],
                           pT2[:, 2:4, :])
            # d-step 3: E(NW-2) halves with the last window's transpose pairs
            # tucked between (copies split DVE/Act so E(NW-1)'s chain starts
            # on its first attnT chunk while the second half is in flight)
            wL = NW - 1
            eoC = eop.tile([P, DIM], BF16, tag="eo", name="eo")
            pTL = psT.tile([P, 4, P], F32, tag="tr", name="pTL")
            pf, emms = emit_E_mms(NW - 2, 0)
            for mm in emms:
                mm()
            for m in (0, 1):
                nc.tensor.matmul(
                    pTL[:, m, :],
                    lhsT=atok_of[wL][:, 2 * m:2 * m + 2, :],
                    rhs=ident_sb[:],
                    start=True, stop=True,
                )
            drain_E_join(NW - 2, 0, eoC, pf)
            pf, emms = emit_E_mms(NW - 2, 1, pool=psS, ptag="sc")
            for mm in emms:
                mm()
            # attnT copies are emitted only after every E(NW-2) matmul: the
            # dep tracker ranges these strided writes coarsely, so earlier
            # emission would falsely stall the nf1 matmuls on them. Both go
            # to DVE, which is idle here while Act carries the eo joins.
            nc.vector.tensor_copy(attnT_sb[:, 0:2, wL * P:(wL + 1) * P], pTL[:, 0:2, :])
            for m in (2, 3):
                nc.tensor.matmul(
                    pTL[:, m, :],
                    lhsT=atok_of[wL][:, 2 * m:2 * m + 2, :],
                    rhs=ident_sb[:],
                    start=True, stop=True,
                )
            nc.vector.tensor_copy(attnT_sb[:, 2:4, wL * P:(wL + 1) * P], pTL[:, 2:4, :])
            drain_E_join(NW - 2, 1, eoC, pf)
            nc.scalar.dma_start(out_d[(NW - 2) * P:(NW - 1) * P, :], eoC[:, :])
            # E(NW-1): the kernel's critical tail, 256-col copy/store slivers
            # on alternating engines and DMA queues
            eoD = eop.tile([P, DIM], BF16, tag="eo", name="eo")
            pf, emms = emit_E_mms(NW - 1, 0)
            for mm in emms:
                mm()
            nc.scalar.copy(eoD[:, 0:256], pf[:, 0:256])
            nc.vector.tensor_copy(eoD[:, 256:512], pf[:, 256:512])
            nc.sync.dma_start(out_d[wL * P:(wL + 1) * P, 0:256], eoD[:, 0:256])
            nc.gpsimd.dma_start(out_d[wL * P:(wL + 1) * P, 256:512], eoD[:, 256:512])
            pf, emms = emit_E_mms(NW - 1, 1, pool=psS, ptag="sc")
            for mm in emms:
                mm()
            nc.scalar.copy(eoD[:, 512:768], pf[:, 0:256])
            nc.vector.tensor_copy(eoD[:, 768:1024], pf[:, 256:512])
            nc.sync.dma_start(out_d[wL * P:(wL + 1) * P, 512:768], eoD[:, 512:768])
            nc.scalar.dma_start(out_d[wL * P:(wL + 1) * P, 768:1024], eoD[:, 768:1024])

    nc.compile()
    return nc


def _get_nc():
    if "nc" not in _CACHE:
        _CACHE["nc"] = _build()
    return _CACHE["nc"]


def make_in_maps(x, w_qkv, w_out):
    x = np.asarray(x, dtype=np.float32)
    w_qkv_b = np.asarray(w_qkv, dtype=np.float32).astype(ml_dtypes.bfloat16)
    w_out_b = np.asarray(w_out, dtype=np.float32).astype(ml_dtypes.bfloat16)

    # mask2[j, c, i] = 1 where j <= i, replicated for both heads of a group
    maskT = np.triu(np.ones((P, P), dtype=np.float32))
    mask2 = np.broadcast_to(maskT[:, None, :], (P, 2, P)).astype(ml_dtypes.bfloat16)
    mask2 = np.ascontiguousarray(mask2)
    ident = np.eye(P, dtype=np.float32).astype(ml_dtypes.bfloat16)

    x_pad = np.concatenate([np.zeros((WSZ, DIM), np.float32), x], axis=0)
    in_maps = []
    for c in range(NCORES):
        x_sh = x_pad[c * R:c * R + T]                       # (2176, 1024)
        xT = np.ascontiguousarray(x_sh.T).astype(ml_dtypes.bfloat16)
        in_maps.append({
            "xT": xT,
            "wqkv": w_qkv_b,
            "wout": w_out_b,
            "mask2": mask2,
            "ident": ident,
        })
    return in_maps


def kernel(x, w_qkv, w_out, b_out):
    b_out = np.asarray(b_out, dtype=np.float32)
    in_maps = make_in_maps(x, w_qkv, w_out)
    nc = _get_nc()
    res = run_bass_kernel_spmd(nc, in_maps, core_ids=list(range(NCORES)))
    out = np.concatenate(
        [res.results[c]["out"].astype(np.float32) for c in range(NCORES)], axis=0
    )
    return out + b_out[None, :]

